# revision 5
# baseline (speedup 1.0000x reference)
"""Trainium2 Bass kernel for nn_CNF (FFJORD-style continuous normalizing flow).

Data-parallel over particles: N=65536 is sharded across 8 NeuronCores
(8192 particles per core).  The tiny hypernet (weights W(t), B(t), U(t)
depend only on `ts`) is evaluated on the host for the 112 fixed RK4
evaluation times; the device kernel does the heavy per-particle work:

  per RHS eval:   a = W z + B        (row-tiled matmuls, K=8)
                  h = tanh(a)        (ScalarE, bias fused)
                  hh = h*h           (VectorE)
                  dz/tr              (col-tiled matmuls, K=128, M=9;
                                      tr rides the M=9 trick: one extra
                                      stationary column = alpha*c writes
                                      the trace row; dz and tr accumulate
                                      into the same PSUM bank)
  RK4 y-builds and the weighted state accumulation are fused
  scalar_tensor_tensor AXPYs with all dt factors folded into the
  host-precomputed stationaries.

On-core layout ("holey"): particles are split into 4 groups x 4 chunks
of 512.  Chunk (g, j) of the state lives at SBUF partitions 32j..32j+7
(rows = the 8 z-components), free = 512 within group tile g; logp lives
at partition 32j+8.  This makes every vector op a dense [128, 512] op
and every matmul a natural row/col tile_position target.
"""
import sys

sys.path.insert(0, "/opt/trn_rl_repo")

import numpy as np

import concourse.bacc as bacc
import concourse.mybir as mybir
from concourse.tile import TileContext
from concourse.bass_utils import run_bass_kernel_spmd

# problem constants (hardcoded per contract)
D = 8
HIDDEN = 64
WIDTH = 128
N = 65536
T = 8
NSUB = 4
BS = WIDTH * D
NCORES = 8
NPC = N // NCORES          # particles per core = 8192
NSEG = T - 1               # 7
NEVAL = NSEG * NSUB * 4    # 112
F = 512                    # particles per chunk
NGRP = 4                   # groups (and row/col tile positions per group)

# per-eval weight block layout in the packed weights tensor (free offsets)
EVW = 128 + 1 + 9 + 9      # Wt4 | B | Sh | Shh  = 147 floats
OFF_BASE = NEVAL * EVW     # 7 per-segment offset columns after the eval blocks
WTS_FREE = OFF_BASE + NSEG

FP32 = mybir.dt.float32
# matmul input dtype: float32r = full fp32 storage, PE runs it at 1 cycle/row
# (vs 4 for float32) with slightly reduced multiply precision.
MM_DT = FP32

_CACHED = {}


def _build_nc():
    nc = bacc.Bacc(None, target_bir_lowering=False)
    state0_d = nc.dram_tensor("state0", [128, NGRP * F], FP32, kind="ExternalInput")
    wts_d = nc.dram_tensor("wts", [128, WTS_FREE], FP32, kind="ExternalInput")
    out_d = nc.dram_tensor("state_out", [NSEG, 128, NGRP * F], FP32,
                           kind="ExternalOutput")

    AF = mybir.ActivationFunctionType
    OP = mybir.AluOpType
    R_S = [1.0 / 3.0, 2.0 / 3.0, 1.0 / 3.0, 1.0]

    with TileContext(nc) as tc:
        with (
            tc.tile_pool(name="wpool", bufs=1) as wpool,
            tc.tile_pool(name="stpool", bufs=1) as stpool,
            tc.tile_pool(name="hpool", bufs=3) as hpool,
            tc.tile_pool(name="ypool", bufs=8) as ypool,
            tc.tile_pool(name="apsum", bufs=1, space="PSUM") as apsum,
            tc.tile_pool(name="dzpsum", bufs=3, space="PSUM") as dzpsum,
        ):
            wts = wpool.tile([128, WTS_FREE], FP32, tag="wts")
            nc.sync.dma_start(out=wts[:], in_=wts_d[:])
            sta = [stpool.tile([128, F], FP32, tag=f"sta{g}", name=f"sta{g}")
                   for g in range(NGRP)]
            stb = [stpool.tile([128, F], FP32, tag=f"stb{g}", name=f"stb{g}")
                   for g in range(NGRP)]
            for g in range(NGRP):
                nc.sync.dma_start(out=sta[g][:],
                                  in_=state0_d[:, g * F:(g + 1) * F])
            tc.strict_bb_all_engine_barrier()

            def r(ap):
                return ap.bitcast(MM_DT) if MM_DT != FP32 else ap

            cur, nxt = sta, stb
            for seg in range(NSEG):
                for sub in range(NSUB):
                    last_sub = sub == NSUB - 1
                    y_prev = [None] * NGRP
                    for s in range(4):
                        e = (seg * NSUB + sub) * 4 + s
                        we = e * EVW
                        for g in range(NGRP):
                            mov = cur[g] if s == 0 else y_prev[g]
                            a_ps = apsum.tile([128, NGRP * F], FP32, tag="a")
                            for j in range(NGRP):
                                nc.tensor.matmul(
                                    a_ps[:, j * F:(j + 1) * F],
                                    lhsT=r(wts[32 * j:32 * j + 8, we:we + 128]),
                                    rhs=r(mov[32 * j:32 * j + 8, :]),
                                    start=True, stop=True,
                                    tile_position=(32 * j, 0),
                                )
                            h_t = hpool.tile([128, NGRP * F], FP32, tag="h")
                            nc.scalar.activation(h_t[:], a_ps[:], AF.Tanh,
                                                 bias=wts[:, we + 128:we + 129],
                                                 scale=1.0)
                            hh_t = hpool.tile([128, NGRP * F], FP32, tag="hh")
                            nc.vector.tensor_tensor(out=hh_t[:], in0=h_t[:],
                                                    in1=h_t[:], op=OP.mult)
                            dz_ps = dzpsum.tile([128, F], FP32, tag="dz")
                            for j in range(NGRP):
                                nc.tensor.matmul(
                                    dz_ps[32 * j:32 * j + 9, :],
                                    lhsT=r(wts[:, we + 138:we + 147]),
                                    rhs=r(hh_t[:, j * F:(j + 1) * F]),
                                    start=True, stop=False,
                                    tile_position=(0, 32 * j),
                                )
                                nc.tensor.matmul(
                                    dz_ps[32 * j:32 * j + 9, :],
                                    lhsT=r(wts[:, we + 129:we + 138]),
                                    rhs=r(h_t[:, j * F:(j + 1) * F]),
                                    start=False, stop=True,
                                    tile_position=(0, 32 * j),
                                )
                            if s < 3:
                                y_t = ypool.tile([128, F], FP32, tag="y")
                                nc.vector.tensor_tensor(out=y_t[:], in0=dz_ps[:],
                                                        in1=cur[g][:], op=OP.add)
                                y_prev[g] = y_t
                            # state accumulation: acc = (psum * r_s) + prev
                            if s == 0:
                                nc.vector.scalar_tensor_tensor(
                                    out=nxt[g][:], in0=dz_ps[:], scalar=R_S[0],
                                    in1=cur[g][:], op0=OP.mult, op1=OP.add)
                            elif s == 3 and last_sub:
                                # fold the segment's -sum(w*sum(c)) logp offset
                                nc.vector.scalar_tensor_tensor(
                                    out=nxt[g][:], in0=dz_ps[:],
                                    scalar=wts[:, OFF_BASE + seg:OFF_BASE + seg + 1],
                                    in1=nxt[g][:], op0=OP.add, op1=OP.add)
                            else:
                                nc.vector.scalar_tensor_tensor(
                                    out=nxt[g][:], in0=dz_ps[:], scalar=R_S[s],
                                    in1=nxt[g][:], op0=OP.mult, op1=OP.add)
                    cur, nxt = nxt, cur
                for g in range(NGRP):
                    nc.sync.dma_start(out=out_d[seg, :, g * F:(g + 1) * F],
                                      in_=cur[g][:])
    nc.finalize()
    return nc


def _hyper_np(t, w1, b1, w2, b2, w3, b3):
    h = np.tanh(w1[:, 0] * np.float32(t) + b1)
    h = np.tanh(w2 @ h + b2)
    p = w3 @ h + b3
    W = p[:BS].reshape(WIDTH, D)
    U = p[BS:2 * BS].reshape(WIDTH, D)
    G = p[2 * BS:3 * BS].reshape(WIDTH, D)
    U = U * (1.0 / (1.0 + np.exp(-G.astype(np.float64)))).astype(np.float32)
    B = p[3 * BS:]
    return W, B, U


def _pack_weights(ts, w1, b1, w2, b2, w3, b3):
    """Host hypernet evaluation + stationary packing for all 112 evals."""
    wts = np.zeros((128, WTS_FREE), np.float32)
    f32 = np.float32
    for seg in range(NSEG):
        t0, t1 = f32(ts[seg]), f32(ts[seg + 1])
        dt = f32((t1 - t0) / NSUB)
        dt2 = f32(dt / 2)
        alphas = [dt2, dt2, dt, f32(dt / 6)]
        wvec = [f32(dt / 6), f32(dt / 3), f32(dt / 3), f32(dt / 6)]
        off = np.float64(0.0)
        t_acc = t0
        for sub in range(NSUB):
            stage_ts = [t_acc, f32(t_acc + dt2), f32(t_acc + dt2), f32(t_acc + dt)]
            for s in range(4):
                e = (seg * NSUB + sub) * 4 + s
                W, B, U = _hyper_np(stage_ts[s], w1, b1, w2, b2, w3, b3)
                c = (W * U).sum(axis=1) / WIDTH          # [128]
                we = e * EVW
                # Wt4: row-tile stationaries, replicated at 4 partition offsets
                for j in range(NGRP):
                    wts[32 * j:32 * j + 8, we:we + 128] = W.T
                wts[:, we + 128] = B
                # Sh: [aU'/128 | 0]  (dz from h)
                wts[:, we + 129:we + 137] = alphas[s] * U / WIDTH
                # Shh: [0 x8 | a*c]  (trace row from hh)
                wts[:, we + 146] = alphas[s] * c
                off += np.float64(wvec[s]) * np.float64(c.sum())
            t_acc = f32(t_acc + dt)
        # logp offset vector: applied once per segment on lp rows (32j+8)
        for j in range(NGRP):
            wts[32 * j + 8, OFF_BASE + seg] = np.float32(-off)
    return wts


def _pack_state0(z0c, lp0c):
    """[8192, 8] + [8192, 1] -> holey [128, 2048]."""
    z = z0c.reshape(NGRP, NGRP, F, D)        # [g, j, f, d]
    lp = lp0c.reshape(NGRP, NGRP, F)         # [g, j, f]
    arr = np.zeros((NGRP, 32, NGRP, F), np.float32)   # [j, row, g, f]
    arr[:, :D] = z.transpose(1, 3, 0, 2)
    arr[:, D] = lp.transpose(1, 0, 2)
    return arr.reshape(128, NGRP * F)


def _unpack_out(out):
    """[7, 128, 2048] -> (z [7, 8192, 8], lp [7, 8192, 1])."""
    v = out.reshape(NSEG, NGRP, 32, NGRP, F)           # [seg, j, row, g, f]
    z = v[:, :, :D].transpose(0, 3, 1, 4, 2).reshape(NSEG, NPC, D)
    lp = v[:, :, D].transpose(0, 2, 1, 3).reshape(NSEG, NPC, 1)
    return z, lp


def kernel(ts, z0, logp_diff_t0, w1, b1, w2, b2, w3, b3, _profile=False):
    ts = np.asarray(ts, np.float32)
    z0 = np.asarray(z0, np.float32)
    lp0 = np.asarray(logp_diff_t0, np.float32)
    w1 = np.asarray(w1, np.float32); b1 = np.asarray(b1, np.float32)
    w2 = np.asarray(w2, np.float32); b2 = np.asarray(b2, np.float32)
    w3 = np.asarray(w3, np.float32); b3 = np.asarray(b3, np.float32)

    wts = _pack_weights(ts, w1, b1, w2, b2, w3, b3)
    in_maps = []
    for c in range(NCORES):
        sl = slice(c * NPC, (c + 1) * NPC)
        in_maps.append({
            "state0": _pack_state0(z0[sl], lp0[sl]),
            "wts": wts,
        })

    if "nc" not in _CACHED:
        _CACHED["nc"] = _build_nc()
    nc = _CACHED["nc"]
    res = run_bass_kernel_spmd(nc, in_maps, list(range(NCORES)),
                               trace=bool(_profile))

    zt = np.empty((T, N, D), np.float32)
    lpt = np.empty((T, N, 1), np.float32)
    zt[0] = z0
    lpt[0] = lp0
    for c in range(NCORES):
        sl = slice(c * NPC, (c + 1) * NPC)
        z, lp = _unpack_out(res.results[c]["state_out"])
        zt[1:, sl] = z
        lpt[1:, sl] = lp
    if _profile:
        return (zt, lpt), res
    return zt, lpt


# revision 8
# speedup vs baseline: 2.8423x; 2.8423x over previous
"""Trainium2 Bass kernel for nn_CNF (FFJORD-style continuous normalizing flow).

Data-parallel over particles: N=65536 is sharded across 8 NeuronCores
(8192 particles per core).  The tiny hypernet (weights W(t), B(t), U(t)
depend only on `ts`) is evaluated on the host for the 112 fixed RK4
evaluation times; the device kernel does the heavy per-particle work:

  per RHS eval:   a = W z + B        (row-tiled matmuls, K=8, bf16)
                  h = tanh(a)        (ScalarE, bias fused, bf16 out)
                  hh = h*h           (VectorE, all-bf16 2x mode)
                  dz/tr              (col-tiled matmuls, K=128, M=9, bf16;
                                      tr rides the M=9 trick: one extra
                                      stationary column = alpha*c writes
                                      the trace row; dz and tr accumulate
                                      into the same PSUM bank)
  RK4 y-builds and the weighted state accumulation are fused fp32
  scalar_tensor_tensor AXPYs (PSUM source) with all dt factors folded
  into the host-precomputed stationaries.  The fp32 state is only ever
  *read* through a bf16 cast as matmul moving data; it accumulates in
  fp32, so bf16 only perturbs the force evaluation (~1e-4 relative on
  the trajectory), not the state arithmetic.

On-core layout ("holey"): particles are split into 2 pairs x 2 groups x
4 chunks of 512.  Chunk (g, j) of the state lives at SBUF partitions
32j..32j+7 (rows = the 8 z-components), free = (g%2)*512 within pair
tile p=g//2; logp lives at partition 32j+8.  Every vector op is a dense
[128, 1024] op and every matmul a natural row/col tile_position target.
"""
import sys

sys.path.insert(0, "/opt/trn_rl_repo")

import numpy as np

import concourse.bacc as bacc
import concourse.mybir as mybir
from concourse.tile import TileContext
from concourse.bass_utils import run_bass_kernel_spmd

# problem constants (hardcoded per contract)
D = 8
HIDDEN = 64
WIDTH = 128
N = 65536
T = 8
NSUB = 4
BS = WIDTH * D
NCORES = 8
NPC = N // NCORES          # particles per core = 8192
NSEG = T - 1               # 7
NEVAL = NSEG * NSUB * 4    # 112
F = 512                    # particles per chunk
NGRP = 4                   # groups (and row/col tile positions)
NPAIR = 2                  # pairs of groups
PF = 2 * F                 # pair free size = 1024

# packed weights layouts
EVWB = 128 + 9             # bf16 per-eval block: Wt4 | Sh
BF_FREE = NEVAL * EVWB
OFF_BASE = NEVAL           # fp32 tensor: B columns then offvec columns
F32_FREE = NEVAL + NSEG

FP32 = mybir.dt.float32
BF16 = mybir.dt.bfloat16
FP16 = mybir.dt.float16
TR_SCALE = 64.0            # tr-mm runs as (64*a*c) @ (h*h/64) to keep fp16 normal

_CACHED = {}


def _build_nc():
    nc = bacc.Bacc(None, target_bir_lowering=False)
    state0_d = nc.dram_tensor("state0", [128, NGRP * F], FP32, kind="ExternalInput")
    wtsb_d = nc.dram_tensor("wtsb", [128, BF_FREE], BF16, kind="ExternalInput")
    wtsh_d = nc.dram_tensor("wtsh", [128, NEVAL * 9], FP16, kind="ExternalInput")
    wts_d = nc.dram_tensor("wts", [128, F32_FREE], FP32, kind="ExternalInput")
    out_d = nc.dram_tensor("state_out", [NSEG, 128, NGRP * F], FP32,
                           kind="ExternalOutput")

    AF = mybir.ActivationFunctionType
    OP = mybir.AluOpType
    R_S = [1.0 / 3.0, 2.0 / 3.0, 1.0 / 3.0, 1.0]

    with TileContext(nc) as tc:
        with (
            tc.tile_pool(name="wpool", bufs=1) as wpool,
            tc.tile_pool(name="stpool", bufs=1) as stpool,
            tc.tile_pool(name="hpool", bufs=2) as hpool,
            tc.tile_pool(name="ypool", bufs=6) as ypool,
            tc.tile_pool(name="apsum", bufs=2, space="PSUM") as apsum,
            tc.tile_pool(name="dzpsum", bufs=2, space="PSUM") as dzpsum,
        ):
            wtsb = wpool.tile([128, BF_FREE], BF16, tag="wtsb")
            wtsh = wpool.tile([128, NEVAL * 9], FP16, tag="wtsh")
            wts = wpool.tile([128, F32_FREE], FP32, tag="wts")
            nc.sync.dma_start(out=wtsb[:], in_=wtsb_d[:])
            nc.sync.dma_start(out=wtsh[:], in_=wtsh_d[:])
            nc.sync.dma_start(out=wts[:], in_=wts_d[:])
            sta = [stpool.tile([128, PF], FP32, tag=f"sta{p}", name=f"sta{p}")
                   for p in range(NPAIR)]
            stb = [stpool.tile([128, PF], FP32, tag=f"stb{p}", name=f"stb{p}")
                   for p in range(NPAIR)]
            for p in range(NPAIR):
                nc.sync.dma_start(out=sta[p][:],
                                  in_=state0_d[:, p * PF:(p + 1) * PF])
            tc.strict_bb_all_engine_barrier()

            cur, nxt = sta, stb
            for seg in range(NSEG):
                for sub in range(NSUB):
                    last_sub = sub == NSUB - 1
                    # bf16 shadow of the substep-start state (stage-0 moving)
                    y0 = []
                    for p in range(NPAIR):
                        y0_t = ypool.tile([128, PF], BF16, tag="y", name=f"y0_{p}")
                        nc.vector.tensor_copy(out=y0_t[:], in_=cur[p][:])
                        y0.append(y0_t)
                    y_prev = y0
                    for s in range(4):
                        e = (seg * NSUB + sub) * 4 + s
                        wb = e * EVWB
                        for p in range(NPAIR):
                            mov = y_prev[p]
                            h_t = hpool.tile([128, NGRP * PF], BF16, tag="h")
                            for j in range(NGRP):
                                a_ps = apsum.tile([128, PF], FP32, tag="a")
                                for q in range(2):
                                    nc.tensor.matmul(
                                        a_ps[:, q * F:(q + 1) * F],
                                        lhsT=wtsb[32 * j:32 * j + 8, wb:wb + 128],
                                        rhs=mov[32 * j:32 * j + 8, q * F:(q + 1) * F],
                                        start=True, stop=True,
                                        tile_position=(32 * j, 0),
                                    )
                                nc.scalar.activation(
                                    h_t[:, j * PF:(j + 1) * PF], a_ps[:], AF.Tanh,
                                    bias=wts[:, e:e + 1], scale=1.0)
                            hh_t = hpool.tile([128, NGRP * PF], FP16, tag="hh")
                            nc.vector.scalar_tensor_tensor(
                                out=hh_t[:], in0=h_t[:], scalar=1.0 / TR_SCALE,
                                in1=h_t[:], op0=OP.mult, op1=OP.mult)
                            dz_ps = dzpsum.tile([128, PF], FP32, tag="dz")
                            for j in range(NGRP):
                                for q in range(2):
                                    nc.tensor.matmul(
                                        dz_ps[32 * j:32 * j + 9, q * F:(q + 1) * F],
                                        lhsT=wtsh[:, e * 9:e * 9 + 9],
                                        rhs=hh_t[:, j * PF + q * F:j * PF + (q + 1) * F],
                                        start=True, stop=False,
                                        tile_position=(0, 32 * j),
                                    )
                                    nc.tensor.matmul(
                                        dz_ps[32 * j:32 * j + 9, q * F:(q + 1) * F],
                                        lhsT=wtsb[:, wb + 128:wb + 137],
                                        rhs=h_t[:, j * PF + q * F:j * PF + (q + 1) * F],
                                        start=False, stop=True,
                                        tile_position=(0, 32 * j),
                                    )
                            if s < 3:
                                y_t = ypool.tile([128, PF], BF16, tag="y")
                                nc.vector.tensor_tensor(out=y_t[:], in0=dz_ps[:],
                                                        in1=cur[p][:], op=OP.add)
                                y_prev[p] = y_t
                            # state accumulation: acc = (psum * r_s) + prev
                            if s == 0:
                                nc.vector.scalar_tensor_tensor(
                                    out=nxt[p][:], in0=dz_ps[:], scalar=R_S[0],
                                    in1=cur[p][:], op0=OP.mult, op1=OP.add)
                            elif s == 3 and last_sub:
                                # fold the segment's -sum(w*sum(c)) logp offset
                                nc.vector.scalar_tensor_tensor(
                                    out=nxt[p][:], in0=dz_ps[:],
                                    scalar=wts[:, OFF_BASE + seg:OFF_BASE + seg + 1],
                                    in1=nxt[p][:], op0=OP.add, op1=OP.add)
                            else:
                                nc.vector.scalar_tensor_tensor(
                                    out=nxt[p][:], in0=dz_ps[:], scalar=R_S[s],
                                    in1=nxt[p][:], op0=OP.mult, op1=OP.add)
                    cur, nxt = nxt, cur
                for p in range(NPAIR):
                    nc.sync.dma_start(out=out_d[seg, :, p * PF:(p + 1) * PF],
                                      in_=cur[p][:])
    nc.finalize()
    return nc


def _hyper_np(t, w1, b1, w2, b2, w3, b3):
    h = np.tanh(w1[:, 0] * np.float32(t) + b1)
    h = np.tanh(w2 @ h + b2)
    p = w3 @ h + b3
    W = p[:BS].reshape(WIDTH, D)
    U = p[BS:2 * BS].reshape(WIDTH, D)
    G = p[2 * BS:3 * BS].reshape(WIDTH, D)
    U = U * (1.0 / (1.0 + np.exp(-G.astype(np.float64)))).astype(np.float32)
    B = p[3 * BS:]
    return W, B, U


def _pack_weights(ts, w1, b1, w2, b2, w3, b3):
    """Host hypernet evaluation + stationary packing for all 112 evals."""
    import ml_dtypes
    wtsb = np.zeros((128, BF_FREE), ml_dtypes.bfloat16)
    wtsh = np.zeros((128, NEVAL * 9), np.float16)
    wts = np.zeros((128, F32_FREE), np.float32)
    f32 = np.float32
    for seg in range(NSEG):
        t0, t1 = f32(ts[seg]), f32(ts[seg + 1])
        dt = f32((t1 - t0) / NSUB)
        dt2 = f32(dt / 2)
        alphas = [dt2, dt2, dt, f32(dt / 6)]
        wvec = [f32(dt / 6), f32(dt / 3), f32(dt / 3), f32(dt / 6)]
        off = np.float64(0.0)
        t_acc = t0
        for sub in range(NSUB):
            stage_ts = [t_acc, f32(t_acc + dt2), f32(t_acc + dt2), f32(t_acc + dt)]
            for s in range(4):
                e = (seg * NSUB + sub) * 4 + s
                W, B, U = _hyper_np(stage_ts[s], w1, b1, w2, b2, w3, b3)
                c = (W * U).sum(axis=1) / WIDTH          # [128]
                wb = e * EVWB
                # Wt4: row-tile stationaries, replicated at 4 partition offsets
                for j in range(NGRP):
                    wtsb[32 * j:32 * j + 8, wb:wb + 128] = W.T
                wts[:, e] = B
                # Sh: [aU'/128 | 0]  (dz from h)
                wtsb[:, wb + 128:wb + 136] = alphas[s] * U / WIDTH
                # Shh: [0 x8 | 64*a*c]  (trace row, fp16, moving is h*h/64)
                wtsh[:, e * 9 + 8] = (TR_SCALE * alphas[s] * c).astype(np.float16)
                off += np.float64(wvec[s]) * np.float64(c.sum())
            t_acc = f32(t_acc + dt)
        # logp offset vector: applied once per segment on lp rows (32j+8)
        for j in range(NGRP):
            wts[32 * j + 8, OFF_BASE + seg] = np.float32(-off)
    return wtsb, wtsh, wts


def _pack_state0(z0c, lp0c):
    """[8192, 8] + [8192, 1] -> holey [128, 2048]."""
    z = z0c.reshape(NGRP, NGRP, F, D)        # [g, j, f, d]
    lp = lp0c.reshape(NGRP, NGRP, F)         # [g, j, f]
    arr = np.zeros((NGRP, 32, NGRP, F), np.float32)   # [j, row, g, f]
    arr[:, :D] = z.transpose(1, 3, 0, 2)
    arr[:, D] = lp.transpose(1, 0, 2)
    return arr.reshape(128, NGRP * F)


def _unpack_out(out):
    """[7, 128, 2048] -> (z [7, 8192, 8], lp [7, 8192, 1])."""
    v = out.reshape(NSEG, NGRP, 32, NGRP, F)           # [seg, j, row, g, f]
    z = v[:, :, :D].transpose(0, 3, 1, 4, 2).reshape(NSEG, NPC, D)
    lp = v[:, :, D].transpose(0, 2, 1, 3).reshape(NSEG, NPC, 1)
    return z, lp


def kernel(ts, z0, logp_diff_t0, w1, b1, w2, b2, w3, b3, _profile=False):
    ts = np.asarray(ts, np.float32)
    z0 = np.asarray(z0, np.float32)
    lp0 = np.asarray(logp_diff_t0, np.float32)
    w1 = np.asarray(w1, np.float32); b1 = np.asarray(b1, np.float32)
    w2 = np.asarray(w2, np.float32); b2 = np.asarray(b2, np.float32)
    w3 = np.asarray(w3, np.float32); b3 = np.asarray(b3, np.float32)

    wtsb, wtsh, wts = _pack_weights(ts, w1, b1, w2, b2, w3, b3)
    in_maps = []
    for c in range(NCORES):
        sl = slice(c * NPC, (c + 1) * NPC)
        in_maps.append({
            "state0": _pack_state0(z0[sl], lp0[sl]),
            "wtsb": wtsb,
            "wtsh": wtsh,
            "wts": wts,
        })

    if "nc" not in _CACHED:
        _CACHED["nc"] = _build_nc()
    nc = _CACHED["nc"]
    res = run_bass_kernel_spmd(nc, in_maps, list(range(NCORES)),
                               trace=bool(_profile))

    zt = np.empty((T, N, D), np.float32)
    lpt = np.empty((T, N, 1), np.float32)
    zt[0] = z0
    lpt[0] = lp0
    for c in range(NCORES):
        sl = slice(c * NPC, (c + 1) * NPC)
        z, lp = _unpack_out(res.results[c]["state_out"])
        zt[1:, sl] = z
        lpt[1:, sl] = lp
    if _profile:
        return (zt, lpt), res
    return zt, lpt
